# revision 1
# baseline (speedup 1.0000x reference)
"""Trainium2 Bass kernel for a w8a16 gated MLP (DeepSeek-style dense MLP).

out = (silu(x @ W0) * (x @ W1)) @ W2,  W* = int8-valued weights * per-128-row-block scales.

Strategy: data-parallel over the 8192 tokens across 8 NeuronCores (1024 tokens/core),
full weights replicated per core. No collectives needed.

Per core:
  phase 1: hT[i, t] = silu(x@W0)^T * (x@W1)^T computed i-tile by i-tile
           (lhsT = W0/W1 column block [128h x 128i], rhs = resident xT [128h x t]),
           PSUM fp32 accumulation over the 32 h-subtiles, SiLU on the scalar engine,
           gate*up on the vector engine, spilled to a DRAM scratch in bf16.
  phase 2: out[t, h'] = hT.T @ W2 streamed over 8 n-panels of 512, 8 PSUM banks
           (one per 128-token m-tile), fp32 accumulation over the 112 i-subtiles.

Host side only reshapes/casts and applies the dequant scales; all matmul FLOPs run
on-device in bf16 (int8 weight values are exact in bf16).
"""

import os

import numpy as np
import ml_dtypes

H = 4096          # hidden
I = 14336         # intermediate
BS = 128          # quant blocksize
B, S = 4, 2048
T_FULL = B * S    # 8192 tokens
N_CORES = 8
T = T_FULL // N_CORES   # 1024 tokens per core

P = 128
KO = H // P       # 32  k-subtiles for phase 1
IT = I // P       # 112 i-tiles (phase-1 output tiles / phase-2 k-subtiles)
TH = T // 512     # 2   token halves for phase-1 psum (N=512 each)
NP = H // 512     # 8   phase-2 n-panels of 512
MT = T // P       # 8   phase-2 m-tiles

BF16 = ml_dtypes.bfloat16
ACT_FN = "Silu"  # sim override hook (CoreSim lacks Silu)
PHASES = (1, 2)  # debug hook: build only selected phases
REPS = 1  # debug hook: on-device repetitions (timing amplification)

_PROGRAM = None
_last_in_maps = None


def _build_program():
    import concourse.mybir as mybir
    from concourse import bacc
    from concourse.tile import TileContext

    bf = mybir.dt.bfloat16
    f32 = mybir.dt.float32

    nc = bacc.Bacc(None, target_bir_lowering=False)

    xt = nc.declare_dram_parameter("xt", [KO, P, T], bf, isOutput=False)
    w0 = nc.declare_dram_parameter("w0t", [IT, P, KO, P], bf, isOutput=False)
    w1 = nc.declare_dram_parameter("w1t", [IT, P, KO, P], bf, isOutput=False)
    w2 = nc.declare_dram_parameter("w2t", [NP, IT, P, 512], bf, isOutput=False)
    out = nc.declare_dram_parameter("out", [T, H], f32, isOutput=True)

    from contextlib import ExitStack, nullcontext

    with TileContext(nc) as tc:
        with (
            tc.tile_pool(name="dram", bufs=1, space="DRAM") as dpool,
            tc.tile_pool(name="xpool", bufs=1) as xpool,
            ExitStack() as _rep_ctx,
        ):
            if REPS > 1:
                _rep_ctx.enter_context(tc.For_i(0, REPS, 1))
            ht = dpool.tile([IT, P, T], bf)

            # resident transposed activations: [p, ko, t]
            xts = xpool.tile([P, KO, T], bf, tag="xts")
            for k in range(KO):
                nc.sync.dma_start(out=xts[:, k, :], in_=xt[k])

            # ---------------- phase 1: gate/up + silu*mul ----------------
            if 1 not in PHASES:
                pass
            else:
             with (
                tc.tile_pool(name="wpool", bufs=3) as wpool,
                tc.tile_pool(name="hpool", bufs=3) as hpool,
                tc.tile_pool(name="spool", bufs=3) as spool,
                tc.tile_pool(name="psum1", bufs=2, space="PSUM") as psum1,
            ):
                for it in range(IT):
                    w0blk = wpool.tile([P, KO, P], bf, tag="w0blk")
                    w1blk = wpool.tile([P, KO, P], bf, tag="w1blk")
                    # split loads so they spread across DMA queues
                    for g in range(4):
                        ks = slice(g * (KO // 4), (g + 1) * (KO // 4))
                        nc.sync.dma_start(out=w0blk[:, ks, :], in_=w0[it, :, ks, :])
                        nc.sync.dma_start(out=w1blk[:, ks, :], in_=w1[it, :, ks, :])

                    psg = [psum1.tile([P, 512], f32, tag=f"pg{th}", name=f"pg{th}") for th in range(TH)]
                    psu = [psum1.tile([P, 512], f32, tag=f"pu{th}", name=f"pu{th}") for th in range(TH)]
                    for k in range(KO):
                        st = k == 0
                        sp = k == KO - 1
                        for th in range(TH):
                            nc.tensor.matmul(
                                psg[th],
                                lhsT=w0blk[:, k, :],
                                rhs=xts[:, k, th * 512:(th + 1) * 512],
                                start=st,
                                stop=sp,
                            )
                        for th in range(TH):
                            nc.tensor.matmul(
                                psu[th],
                                lhsT=w1blk[:, k, :],
                                rhs=xts[:, k, th * 512:(th + 1) * 512],
                                start=st,
                                stop=sp,
                            )

                    ht_sb = hpool.tile([P, T], bf, tag="ht_sb")
                    for th in range(TH):
                        sg = spool.tile([P, 512], bf, tag="sg")
                        nc.scalar.activation(
                            sg, psg[th], getattr(mybir.ActivationFunctionType, ACT_FN)
                        )
                        nc.vector.tensor_mul(
                            out=ht_sb[:, th * 512:(th + 1) * 512],
                            in0=sg,
                            in1=psu[th],
                        )
                    for g in range(2):
                        ts_ = slice(g * (T // 2), (g + 1) * (T // 2))
                        nc.sync.dma_start(out=ht[it, :, ts_], in_=ht_sb[:, ts_])

            # ---------------- phase 2: down projection ----------------
            if 2 not in PHASES:
                pass
            else:
             with (
                tc.tile_pool(name="h2pool", bufs=6) as h2pool,
                tc.tile_pool(name="w2pool", bufs=6) as w2pool,
                tc.tile_pool(name="opool", bufs=4) as opool,
                tc.tile_pool(name="psum2", bufs=1, space="PSUM") as psum2,
            ):
                for n in range(NP):
                    pos = [psum2.tile([P, 512], f32, tag=f"po{m}", name=f"po{m}") for m in range(MT)]
                    for k in range(IT):
                        htr = h2pool.tile([P, T], bf, tag="htr")
                        for g in range(2):
                            ts_ = slice(g * (T // 2), (g + 1) * (T // 2))
                            nc.sync.dma_start(out=htr[:, ts_], in_=ht[k, :, ts_])
                        w2b = w2pool.tile([P, 512], bf, tag="w2b")
                        nc.sync.dma_start(out=w2b, in_=w2[n, k])
                        st = k == 0
                        sp = k == IT - 1
                        for m in range(MT):
                            nc.tensor.matmul(
                                pos[m],
                                lhsT=htr[:, m * P:(m + 1) * P],
                                rhs=w2b,
                                start=st,
                                stop=sp,
                            )
                    for m in range(MT):
                        osb = opool.tile([P, 512], f32, tag="osb")
                        nc.vector.tensor_copy(out=osb, in_=pos[m])
                        nc.sync.dma_start(
                            out=out[m * P:(m + 1) * P, n * 512:(n + 1) * 512],
                            in_=osb,
                        )

    nc.compile()
    return nc


def _dequant_bf16(w_int: np.ndarray, s: np.ndarray) -> np.ndarray:
    # w_int [in, out] int32 (int8-valued), s [in//BS, out] fp32 -> bf16 [in, out]
    return (
        w_int.astype(np.float32) * np.repeat(s.astype(np.float32), BS, axis=0)
    ).astype(BF16)


def kernel(x, w0, w1, w2, s0, s1, s2, blocksize):
    global _PROGRAM
    from concourse.bass_utils import run_bass_kernel_spmd

    assert int(blocksize) == BS

    W0 = _dequant_bf16(np.asarray(w0), np.asarray(s0))  # [H, I]
    W1 = _dequant_bf16(np.asarray(w1), np.asarray(s1))  # [H, I]
    W2 = _dequant_bf16(np.asarray(w2), np.asarray(s2))  # [I, H]

    # tiled layouts so every device DMA is contiguous per partition
    w0t = np.ascontiguousarray(W0.reshape(KO, P, IT, P).transpose(2, 1, 0, 3))
    w1t = np.ascontiguousarray(W1.reshape(KO, P, IT, P).transpose(2, 1, 0, 3))
    w2t = np.ascontiguousarray(W2.reshape(IT, P, NP, 512).transpose(2, 0, 1, 3))

    x_flat = np.asarray(x, dtype=np.float32).reshape(T_FULL, H)

    in_maps = []
    for c in range(N_CORES):
        xs = x_flat[c * T:(c + 1) * T]                     # [T, H]
        xt_c = np.ascontiguousarray(xs.T).astype(BF16).reshape(KO, P, T)
        in_maps.append({"xt": xt_c, "w0t": w0t, "w1t": w1t, "w2t": w2t})

    global _last_in_maps
    _last_in_maps = in_maps
    if _PROGRAM is None:
        _PROGRAM = _build_program()

    trace = os.environ.get("KERNEL_TRACE") == "1"
    if trace:
        try:
            from antenv.axon_hooks import get_axon_ntff_profile_hook  # noqa: F401
        except ImportError:
            trace = False
    r = run_bass_kernel_spmd(_PROGRAM, in_maps, list(range(N_CORES)), trace=trace)
    if trace and r.exec_time_ns is not None:
        print(f"HW exec time: {r.exec_time_ns} ns")
    res = r.results
    out = np.concatenate([np.asarray(res[c]["out"]) for c in range(N_CORES)], axis=0)
    return out.reshape(B, S, H).astype(np.float32)



# revision 2
# speedup vs baseline: 45.9751x; 45.9751x over previous
"""Trainium2 Bass kernel for a w8a16 gated MLP (DeepSeek-style dense MLP).

out = (silu(x @ W0) * (x @ W1)) @ W2,  W* = int8-valued weights * per-128-row-block scales.

Strategy: data-parallel over the 8192 tokens across 8 NeuronCores (1024 tokens/core),
full weights replicated per core. No collectives needed.

Per core:
  phase 1: hT[i, t] = silu(x@W0)^T * (x@W1)^T computed i-tile by i-tile
           (lhsT = W0/W1 column block [128h x 128i], rhs = resident xT [128h x 512t]),
           PSUM fp32 accumulation over the 32 h-subtiles, SiLU on the scalar engine,
           gate*up on the vector engine, spilled to a DRAM scratch in bf16
           (packed in groups of 4 i-tiles so phase 2 reloads are 1 MB each).
  phase 2: outT[h, t] = W2^T-tile-stationary matmuls with hT streamed as the moving
           operand: for each group of 4 h-tiles (512 output features), accumulate
           over the 112 i-subtiles, each W2 tile [128i x 128h] feeding two N=512
           matmuls (token halves). 8 PSUM banks; fp32 result stored transposed
           [H, T]; the host untransposes.

This keeps the HWDGE DMA-instruction count low (~850 per core, all transfers with
>=1KB contiguous runs) so descriptor generation never starves the tensor engine.

Host side only reshapes/casts and applies the dequant scales; all matmul FLOPs run
on-device in bf16 (int8 weight values are exact in bf16).
"""

import os

import numpy as np
import ml_dtypes

H = 4096          # hidden
I = 14336         # intermediate
BS = 128          # quant blocksize
B, S = 4, 2048
T_FULL = B * S    # 8192 tokens
N_CORES = 8
T = T_FULL // N_CORES   # 1024 tokens per core

P = 128
KO = H // P       # 32  k-subtiles for phase 1
IT = I // P       # 112 i-tiles (phase-1 output tiles / phase-2 k-subtiles)
TH = T // 512     # 2   token halves (N=512 matmuls)
KB = 4            # phase-2 k-batch (i-tiles per DMA)
KG = IT // KB     # 28  k-groups
HG = H // 512     # 8   phase-2 output groups of 512 features (4 h-tiles)

BF16 = ml_dtypes.bfloat16
ACT_FN = "Silu"  # sim override hook (CoreSim lacks Silu)
PHASES = (1, 2)  # debug hook: build only selected phases
REPS = 1  # debug hook: on-device repetitions (timing amplification)

_PROGRAM = None
_last_in_maps = None


def _build_program():
    import concourse.mybir as mybir
    from concourse import bacc
    from concourse.tile import TileContext

    bf = mybir.dt.bfloat16
    f32 = mybir.dt.float32

    nc = bacc.Bacc(None, target_bir_lowering=False)

    xt = nc.declare_dram_parameter("xt", [KO, P, T], bf, isOutput=False)
    w0 = nc.declare_dram_parameter("w0t", [IT, P, KO * P], bf, isOutput=False)
    w1 = nc.declare_dram_parameter("w1t", [IT, P, KO * P], bf, isOutput=False)
    # w2t[hg, kg, p_i, kb, 512h]: element (i = (kg*KB+kb)*P + p_i, h = hg*512 + col)
    w2 = nc.declare_dram_parameter("w2t", [HG, KG, P, KB, 512], bf, isOutput=False)
    # transposed output [H, T]; host untransposes
    out = nc.declare_dram_parameter("out", [H, T], f32, isOutput=True)

    from contextlib import ExitStack

    with TileContext(nc) as tc:
        with (
            tc.tile_pool(name="dram", bufs=1, space="DRAM") as dpool,
            ExitStack() as _rep_ctx,
        ):
            if REPS > 1:
                _rep_ctx.enter_context(tc.For_i(0, REPS, 1))
            # DRAM scratch for hT, packed for 1MB phase-2 loads:
            # ht[kg, p_i, kb, t]  (i-tile it = kg*KB + kb)
            ht = dpool.tile([KG, P, KB, T], bf)

            # ---------------- phase 1: gate/up + silu*mul ----------------
            if 1 in PHASES:
             with (
                tc.tile_pool(name="xpool", bufs=1) as xpool,
                tc.tile_pool(name="wpool", bufs=3) as wpool,
                tc.tile_pool(name="hpool", bufs=3) as hpool,
                tc.tile_pool(name="spool", bufs=3) as spool,
                tc.tile_pool(name="psum1", bufs=2, space="PSUM") as psum1,
            ):
                # resident transposed activations: [p, ko, t]
                xts = xpool.tile([P, KO, T], bf, tag="xts")
                for k in range(KO):
                    nc.sync.dma_start(out=xts[:, k, :], in_=xt[k])

                for it in range(IT):
                    w0blk = wpool.tile([P, KO, P], bf, tag="w0blk")
                    w1blk = wpool.tile([P, KO, P], bf, tag="w1blk")
                    nc.sync.dma_start(out=w0blk[:, :, :], in_=w0[it])
                    nc.sync.dma_start(out=w1blk[:, :, :], in_=w1[it])

                    psg = [psum1.tile([P, 512], f32, tag=f"pg{th}", name=f"pg{th}") for th in range(TH)]
                    psu = [psum1.tile([P, 512], f32, tag=f"pu{th}", name=f"pu{th}") for th in range(TH)]
                    for k in range(KO):
                        st = k == 0
                        sp = k == KO - 1
                        for th in range(TH):
                            nc.tensor.matmul(
                                psg[th],
                                lhsT=w0blk[:, k, :],
                                rhs=xts[:, k, th * 512:(th + 1) * 512],
                                start=st,
                                stop=sp,
                            )
                        for th in range(TH):
                            nc.tensor.matmul(
                                psu[th],
                                lhsT=w1blk[:, k, :],
                                rhs=xts[:, k, th * 512:(th + 1) * 512],
                                start=st,
                                stop=sp,
                            )

                    ht_sb = hpool.tile([P, T], bf, tag="ht_sb")
                    for th in range(TH):
                        sg = spool.tile([P, 512], bf, tag="sg")
                        nc.scalar.activation(
                            sg, psg[th], getattr(mybir.ActivationFunctionType, ACT_FN)
                        )
                        nc.vector.tensor_mul(
                            out=ht_sb[:, th * 512:(th + 1) * 512],
                            in0=sg,
                            in1=psu[th],
                        )
                    nc.sync.dma_start(out=ht[it // KB, :, it % KB, :], in_=ht_sb)

            # ---------------- phase 2: down projection (transposed out) ----------------
            if 2 in PHASES:
             with (
                tc.tile_pool(name="h2pool", bufs=3) as h2pool,
                tc.tile_pool(name="w2pool", bufs=3) as w2pool,
                tc.tile_pool(name="opool", bufs=4) as opool,
                tc.tile_pool(name="psum2", bufs=1, space="PSUM") as psum2,
            ):
                for hg in range(HG):
                    # pos[j][th]: out^T tile [128h, 512t] for h-tile j, token half th
                    pos = [
                        [psum2.tile([P, 512], f32, tag=f"po{j}_{th}", name=f"po{j}_{th}")
                         for th in range(TH)]
                        for j in range(KB)
                    ]
                    for kg in range(KG):
                        htk = h2pool.tile([P, KB, T], bf, tag="htk")
                        nc.sync.dma_start(out=htk[:, :, :], in_=ht[kg])
                        w2b = w2pool.tile([P, KB, 512], bf, tag="w2b")
                        nc.sync.dma_start(out=w2b[:, :, :], in_=w2[hg, kg])
                        for kb in range(KB):
                            st = kg == 0 and kb == 0
                            sp = kg == KG - 1 and kb == KB - 1
                            for j in range(KB):
                                for th in range(TH):
                                    nc.tensor.matmul(
                                        pos[j][th],
                                        lhsT=w2b[:, kb, j * P:(j + 1) * P],
                                        rhs=htk[:, kb, th * 512:(th + 1) * 512],
                                        start=st,
                                        stop=sp,
                                    )
                    for j in range(KB):
                        osb = opool.tile([P, T], f32, tag="osb")
                        for th in range(TH):
                            nc.vector.tensor_copy(out=osb[:, th * 512:(th + 1) * 512], in_=pos[j][th])
                        nc.sync.dma_start(
                            out=out[hg * 512 + j * P: hg * 512 + (j + 1) * P, :],
                            in_=osb,
                        )

    nc.compile()
    return nc


def _dequant_bf16(w_int: np.ndarray, s: np.ndarray) -> np.ndarray:
    # w_int [in, out] int32 (int8-valued), s [in//BS, out] fp32 -> bf16 [in, out]
    return (
        w_int.astype(np.float32) * np.repeat(s.astype(np.float32), BS, axis=0)
    ).astype(BF16)


def kernel(x, w0, w1, w2, s0, s1, s2, blocksize):
    global _PROGRAM
    from concourse.bass_utils import run_bass_kernel_spmd

    assert int(blocksize) == BS

    W0 = _dequant_bf16(np.asarray(w0), np.asarray(s0))  # [H, I]
    W1 = _dequant_bf16(np.asarray(w1), np.asarray(s1))  # [H, I]
    W2 = _dequant_bf16(np.asarray(w2), np.asarray(s2))  # [I, H]

    # tiled layouts so every device DMA has long contiguous per-partition runs
    w0t = np.ascontiguousarray(
        W0.reshape(KO, P, IT, P).transpose(2, 1, 0, 3).reshape(IT, P, KO * P)
    )
    w1t = np.ascontiguousarray(
        W1.reshape(KO, P, IT, P).transpose(2, 1, 0, 3).reshape(IT, P, KO * P)
    )
    # W2[i, h] -> w2t[hg, kg, p_i, kb, col]; i = (kg*KB+kb)*P + p_i, h = hg*512+col
    w2t = np.ascontiguousarray(
        W2.reshape(KG, KB, P, HG, 512).transpose(3, 0, 2, 1, 4)
    )

    x_flat = np.asarray(x, dtype=np.float32).reshape(T_FULL, H)

    in_maps = []
    for c in range(N_CORES):
        xs = x_flat[c * T:(c + 1) * T]                     # [T, H]
        xt_c = np.ascontiguousarray(xs.T).astype(BF16).reshape(KO, P, T)
        in_maps.append({"xt": xt_c, "w0t": w0t, "w1t": w1t, "w2t": w2t})

    global _last_in_maps
    _last_in_maps = in_maps
    if _PROGRAM is None:
        _PROGRAM = _build_program()

    trace = os.environ.get("KERNEL_TRACE") == "1"
    if trace:
        try:
            from antenv.axon_hooks import get_axon_ntff_profile_hook  # noqa: F401
        except ImportError:
            trace = False
    r = run_bass_kernel_spmd(_PROGRAM, in_maps, list(range(N_CORES)), trace=trace)
    if trace and r.exec_time_ns is not None:
        print(f"HW exec time: {r.exec_time_ns} ns")
    res = r.results
    # outputs are [H, T] per core; untranspose and stitch tokens
    out = np.concatenate(
        [np.asarray(res[c]["out"]).T for c in range(N_CORES)], axis=0
    )
    return out.reshape(B, S, H).astype(np.float32)
